# revision 21
# baseline (speedup 1.0000x reference)
"""Trainium2 Bass kernel for nn_LinearPredictionHead (moe_routing).

Reference computation:
    out_e = xs_e[:, :, -1, :] @ W_e + b_e            # [B,C,720] per expert
    combined = sum_e gates[:, e, None] * exp(out_e)  # [B,C,720]
    out = log(max(combined, eps)).transpose(0, 2, 1) # [B,720,C]

Sharding (8 cores, no collectives): 2D data-parallel.
  - B=64 split 4 ways (16 batches -> 512 rows of x per core)
  - P=720 split 2 ways (360 output cols -> W cols per core)
  core c: ib = c // 2 (batch group), ip = c % 2 (p half).

Per-core device kernel (p-major, mixed-precision fp8):
  The rel-err gate is 2e-2; all-e4m3 DoubleRow measures 2.2e-2 and all-e3m4
  measures 1.1e-2 (bit-exact host sim; inputs are deterministic).  The mix
  k[0:512) in e4m3 *DoubleRow* (2 passes of K=256 at 2 fp8/cycle) plus
  k[512:1024) in e3m4 (4-mantissa fp8, bf16-speed) lands at 1.75e-2 with
  72 N=512 matmuls/core instead of 96 bf16 ones, and 1-byte input DMA:
    psum[p, r] = sum_k W32[k, p] * x[k, r]    (W pre-scaled by 32; x as-is)
    psum      += b32[p] x 1[r] + 1[p] x lng32[r]   one K=2 fp16 rank-2 MM
                                              (fold bias AND ln(gate): the
                                               exp then needs no bias AP and
                                               no per-expert DVE multiply)
    te  = exp(psum / 32)                      ACT, one wide [128,1536] call
                                              per expert (3 PSUM banks)
    acc += te                                 DVE wide fp16 add
    out = ln(acc) per p-tile, fp16, DMA'd as each tile finalizes.

  Inputs ship as ONE u8 dram block per expert with 7040B contiguous per
  partition (w-e4m3 | x-e4m3 | w-e3m4 | x-e3m4), one dma_start each
  (~0.88MB at near-peak descriptor efficiency); e0's block is split in two
  so its DoubleRow passes start as early as possible.  Framework trims
  carried over from the previous session: combined exp/ln ACT table, the
  init-time all-engine barrier skip, and the slim TileContext exit.
"""

import os
import sys

import numpy as np

if "/opt/trn_rl_repo" not in sys.path:
    sys.path.insert(0, "/opt/trn_rl_repo")

import ml_dtypes

B, C, E = 64, 32, 4
D, P = 1024, 720
NCORES = 8
BSPLIT, PSPLIT = 4, 2
RB = B // BSPLIT  # 16 batches per core
R = RB * C  # 512 rows per core
PP = P // PSPLIT  # 360 output cols per core
PTS = [(0, 128), (128, 128), (256, 104)]  # p-tiles within PP
SCALE = 32.0  # shared psum scale: W quantized as 32*W, x as-is
WPAD = 368  # W free-dim padded so the DoubleRow pair-step is %16
KDR = 512  # k[0:512) via e4m3 DoubleRow, k[512:1024) via e3m4
# per-partition byte offsets inside one expert's combined input block:
# [wdr-kd0 | xdr-kd0 | wdr-kd1 | xdr-kd1 | we3 | xe3] so a kd-granular
# prefix of the block is already usable by the PE (e0 is DMA'd in 3 pieces).
WKD = 2 * WPAD  # 736 one DoubleRow pass of W pairs
XKD = 2 * R  # 1024 one DoubleRow pass of x pairs
OFF_KD = [0, WKD + XKD]  # kd pass bases (w then x inside each)
OFF_WE3 = 2 * (WKD + XKD)  # 3520
OFF_XE3 = OFF_WE3 + 4 * WPAD  # -> 4992
BLK = OFF_XE3 + 4 * R  # [4c,512] e3m4 = 2048 -> 7040
NWARM = 2

_CACHE = {}
LAST_RESULT = None


def _build_nc():
    import concourse.bass as bass_mod
    import concourse.tile as tile
    from concourse import bacc, mybir

    u8 = mybir.dt.uint8
    f16, f32 = mybir.dt.float16, mybir.dt.float32
    f8e4, f8e3 = mybir.dt.float8e4, mybir.dt.float8e3
    DR = mybir.MatmulPerfMode.DoubleRow
    Exp = mybir.ActivationFunctionType.Exp
    Ln = mybir.ActivationFunctionType.Ln

    # Force Exp and Ln onto the combined act-table set so the kernel loads
    # ONE table instead of reloading on every Exp<->Ln switch.
    import concourse.bacc as bacc_mod
    from concourse.hw_specs import get_activation_tables as _orig_gat

    def _patched_gat(arch):
        tables = _orig_gat(arch)
        for name, funcs in tables.items():
            if name != "natural_log_exp_and_others":
                funcs.discard(mybir.ActivationFunctionType.Exp)
                funcs.discard(mybir.ActivationFunctionType.Ln)
        return tables

    bacc_mod.get_activation_tables = _patched_gat

    # Skip the init-time all-engine barrier: it makes every queue wait for
    # the slowest engine preamble before the first user instruction.
    # Nothing emitted before user code (const-AP memsets on gpsimd) is read
    # by this kernel until the Ln (const 0.0 bias) long after; safe here.
    _orig_aeb = bass_mod.Bass.all_engine_barrier
    _state = {"skipped": False}

    def _patched_aeb(self, *a, **k):
        if not _state["skipped"]:
            _state["skipped"] = True
            return
        return _orig_aeb(self, *a, **k)

    bass_mod.Bass.all_engine_barrier = _patched_aeb
    try:
        nc = bacc.Bacc(
            "TRN2", target_bir_lowering=False, debug=False, num_devices=NCORES
        )
    finally:
        bass_mod.Bass.all_engine_barrier = _orig_aeb

    # TileContext exit: drop the exit barrier AND the framework sem clears.
    # The NEFF-load postamble (runtime-injected) starts with its own entry
    # barrier and then resets the whole sem file, so our exit barrier and
    # clears are pure duplication.
    _orig_dab = tile.TileContext._drain_and_barrier

    def _noexit_dab(self, tick_clock, wait_clock):
        # No completion waits either: the output DMAs land during the
        # ~7us runtime postamble (barrier + 250 sem clears), long before
        # the NEFF's done-notify; the postamble entry barrier then fires
        # as soon as each engine's queue drains.
        popped = self.nc._tile_sem_poison_stack.pop()
        assert popped is self._sem_poison

    tile.TileContext._drain_and_barrier = _noexit_dab
    # (Measured: the postamble begins with its own entry barrier, so the
    # clears cannot overlap the kernel; dropping our exit barrier still
    # saves its sem round-trips.)

    # Host-pretiled inputs: one combined block per expert, 7040B/partition
    # contiguous runs; fp16 rank-2 operands (32*b | ones || ones | 32*lng).
    ixd = nc.dram_tensor("ixd", [E, 128, BLK], u8, kind="ExternalInput").ap()
    blg = nc.dram_tensor("blg", [2, E * (WPAD + R)], f16, kind="ExternalInput").ap()
    out = nc.dram_tensor("out", [3, 128, R], f16, kind="ExternalOutput").ap()
    GL0 = E * WPAD  # column where the gl (rhs) rows start inside blg

    with tile.TileContext(nc) as tc:
        with (
            tc.tile_pool(name="const", bufs=1) as cpool,
            tc.tile_pool(name="psum", bufs=7, space="PSUM") as pspool,
            tc.tile_pool(name="warmps", bufs=1, space="PSUM") as wpool,
            tc.tile_pool(name="texp", bufs=3) as tpool,
            tc.tile_pool(name="lnp", bufs=3) as lnpool,
        ):
            warm_t = cpool.tile([128, 512], f16, tag="warm_t")
            nc.vector.memset(warm_t[:], 0.125)

            inb = [
                cpool.tile([128, BLK], u8, tag=f"in{e}", name=f"in{e}")
                for e in range(E)
            ]
            blg_t = cpool.tile([2, E * (WPAD + R)], f16, tag="blg")
            acc = cpool.tile([128, 3 * 512], f16, tag="acc", name="acc")

            # Scalar (ACT) HWDGE ring dispatches first (its queue reaches
            # user code ~0.9us before sync's): the tiny rank-2 operands
            # lead (they feed the warm-window group openers), then e0's
            # first DoubleRow piece.
            nc.scalar.dma_start(blg_t[:], blg[:, :])
            nc.scalar.dma_start(inb[0][:, : OFF_KD[1]], ixd[0, :, : OFF_KD[1]])
            # Main stream on the sync ring in need-order; e0's remainder in
            # two pieces (kd1 | e3m4).
            nc.sync.dma_start(
                inb[0][:, OFF_KD[1] : OFF_WE3], ixd[0, :, OFF_KD[1] : OFF_WE3]
            )
            nc.sync.dma_start(inb[0][:, OFF_WE3:], ixd[0, :, OFF_WE3:])
            for e in range(1, E):
                nc.sync.dma_start(inb[e][:], ixd[e])

            # fp8 views into the combined blocks
            wdr, xdr, we3, xe3 = [], [], [], []
            for e in range(E):
                wdr.append(
                    [
                        inb[e][:, OFF_KD[kd] : OFF_KD[kd] + WKD]
                        .bitcast(f8e4)
                        .rearrange("p (i w) -> p i w", i=2)
                        for kd in range(2)
                    ]
                )
                xdr.append(
                    [
                        inb[e][:, OFF_KD[kd] + WKD : OFF_KD[kd] + WKD + XKD]
                        .bitcast(f8e4)
                        .rearrange("p (i r) -> p i r", i=2)
                        for kd in range(2)
                    ]
                )
                we3.append(
                    inb[e][:, OFF_WE3:OFF_XE3]
                    .bitcast(f8e3)
                    .rearrange("p (c w) -> p c w", c=4)
                )
                xe3.append(
                    inb[e][:, OFF_XE3:]
                    .bitcast(f8e3)
                    .rearrange("p (c r) -> p c r", c=4)
                )

            # PE warm-up: dep-free matmuls bridge dispatch->first-data so the
            # HAM clock gate opens before real work lands.
            warm_ps = wpool.tile([128, 512], f32, tag="warm")
            for _ in range(NWARM):
                nc.tensor.matmul(
                    warm_ps[:, :], warm_t[:, :128], warm_t[:, :], start=True, stop=True
                )

            def dr_mm(e, ps, kd, p0, plen, start):
                nc.tensor.matmul(
                    ps[:plen, :],
                    wdr[e][kd][:, :, p0 : p0 + plen],
                    xdr[e][kd][:, :, :],
                    start=start,
                    stop=False,
                    perf_mode=DR,
                )

            def e3_mms(e, ps, p0, plen):
                for c in range(4):
                    nc.tensor.matmul(
                        ps[:plen, :],
                        we3[e][:, c, p0 : p0 + plen],
                        xe3[e][:, c, :],
                        start=False,
                        stop=(c == 3),
                    )

            def rank2(e, ps, plen, p0):
                # psum = b32[p] (x) 1[r] + 1[p] (x) lng32[r]: OPENS the
                # accumulation group — only the tiny blg row gates it, so
                # these run inside the pre-data warm window.
                nc.tensor.matmul(
                    ps[:plen, :],
                    blg_t[:, e * WPAD + p0 : e * WPAD + p0 + plen],
                    blg_t[:, GL0 + e * R : GL0 + (e + 1) * R],
                    start=True,
                    stop=False,
                )

            inv = 1.0 / SCALE

            def chain(e, ps, pt):
                # per-p-tile epilogue; single-bank psum tiles keep the next
                # group's matmuls independent of this read.
                sl = slice(512 * pt, 512 * pt + 512)
                if e == 0:
                    nc.scalar.activation(acc[:, sl], ps[:, :], Exp, scale=inv)
                    return
                te = tpool.tile([128, 512], f16, tag="te", name="te")
                nc.scalar.activation(te[:, :], ps[:, :], Exp, scale=inv)
                nc.vector.tensor_add(acc[:, sl], acc[:, sl], te[:, :])
                if e == E - 1:
                    ln_t = lnpool.tile([128, 512], f16, tag="ln")
                    nc.scalar.activation(ln_t[:, :], acc[:, sl], Ln)
                    # stores on the sync ring: ACT's queue then ends at the
                    # last Ln, entering its postamble sooner.
                    nc.sync.dma_start(out[pt], ln_t[:, :])

            # Open the first 7 groups (e0/e1 all p-tiles + e2-pt0 — exactly
            # the psum pool depth) with their rank-2 matmuls during the
            # warm window: only the tiny blg row gates them, they keep the
            # HAM busy, and they drop 7 matmuls out of the loaded stream.
            # (Opening more would deadlock: group 8's opener would block
            # the PE FIFO on a psum buffer freed only by matmuls queued
            # behind it.)
            GRP = [(e, pt) for e in range(E) for pt in range(3)]
            ps_t = {}
            for e, pt in GRP[:7]:
                p0, plen = PTS[pt]
                ps_t[(e, pt)] = pspool.tile([128, 512], f32, tag="ps", name="ps")
                rank2(e, ps_t[(e, pt)], plen, p0)
            # e0: kd-major so only the first ~220KB piece gates its start.
            for kd in range(2):
                for pt, (p0, plen) in enumerate(PTS):
                    dr_mm(0, ps_t[(0, pt)], kd, p0, plen, start=False)
            for pt, (p0, plen) in enumerate(PTS):
                e3_mms(0, ps_t[(0, pt)], p0, plen)
                chain(0, ps_t[(0, pt)], pt)
            # e1..e3: p-tile-sequential groups, each chained immediately.
            for e in range(1, E):
                for pt, (p0, plen) in enumerate(PTS):
                    ps = ps_t.get((e, pt))
                    if ps is None:
                        ps = pspool.tile([128, 512], f32, tag="ps", name="ps")
                        rank2(e, ps, PTS[pt][1], PTS[pt][0])
                    dr_mm(e, ps, 0, p0, plen, start=False)
                    dr_mm(e, ps, 1, p0, plen, start=False)
                    e3_mms(e, ps, p0, plen)
                    chain(e, ps, pt)

    tile.TileContext._drain_and_barrier = _orig_dab
    nc.compile()
    return nc


def _q4(v):
    return np.clip(v, -240.0, 240.0).astype(ml_dtypes.float8_e4m3)


def _q3(v):
    return np.clip(v, -15.5, 15.5).astype(ml_dtypes.float8_e3m4)


def _prep_inputs(inputs):
    gates = np.asarray(inputs["gates"], dtype=np.float64)

    # Per p-half, per expert: W byte blocks [128, 1472] (e4m3 DR) and
    # [128, 1472] (e3m4), plus the fp16 rank-2 lhsT rows.
    w_blocks = []  # [ip][e] -> (wdr_bytes, we3_bytes)
    b_rows = []  # [ip] -> [E*WPAD] fp16 row of 32*b
    for ip in range(PSPLIT):
        per_e = []
        brow = np.zeros(E * WPAD, np.float16)
        for e in range(E):
            W32 = (
                np.asarray(inputs[f"W{e}"][:, ip * PP : (ip + 1) * PP], np.float32)
                * SCALE
            )
            wdr = np.zeros((128, 2, 2, WPAD), ml_dtypes.float8_e4m3)
            wdr[:, :, :, :PP] = _q4(
                W32[:KDR].reshape(2, 2, 128, PP).transpose(2, 0, 1, 3)
            )
            we3 = np.zeros((128, 4, WPAD), ml_dtypes.float8_e3m4)
            we3[:, :, :PP] = _q3(W32[KDR:].reshape(4, 128, PP).transpose(1, 0, 2))
            per_e.append(
                (
                    wdr.view(np.uint8).reshape(128, -1),
                    we3.view(np.uint8).reshape(128, -1),
                )
            )
            brow[e * WPAD : e * WPAD + PP] = (
                SCALE * np.asarray(inputs[f"b{e}"][ip * PP : (ip + 1) * PP])
            ).astype(np.float16)
        w_blocks.append(per_e)
        b_rows.append(brow)

    # Per b-group, per expert: x byte blocks and the lng rank-2 rhs rows.
    x_blocks = []  # [ib][e] -> (xdr_bytes, xe3_bytes)
    g_rows = []  # [ib] -> [E*R] fp16 row of 32*ln(g)
    for ib in range(BSPLIT):
        per_e = []
        grow = np.empty(E * R, np.float16)
        for e in range(E):
            xl = np.asarray(
                inputs[f"xs{e}"][ib * RB : (ib + 1) * RB, :, -1, :], np.float32
            ).reshape(R, D)
            xdr = _q4(xl[:, :KDR].reshape(R, 2, 2, 128).transpose(3, 1, 2, 0))
            xe3 = _q3(xl[:, KDR:].reshape(R, 4, 128).transpose(2, 1, 0))
            per_e.append(
                (
                    np.ascontiguousarray(xdr).view(np.uint8).reshape(128, -1),
                    np.ascontiguousarray(xe3).view(np.uint8).reshape(128, -1),
                )
            )
            lng = SCALE * np.log(np.maximum(gates[ib * RB : (ib + 1) * RB, e], 1e-6))
            grow[e * R : (e + 1) * R] = np.repeat(lng, C).astype(np.float16)
        x_blocks.append(per_e)
        g_rows.append(grow)

    in_maps = []
    for c in range(NCORES):
        ib, ip = divmod(c, PSPLIT)
        ixd = np.empty((E, 128, BLK), np.uint8)
        for e in range(E):
            wdr_b, we3_b = w_blocks[ip][e]  # wdr_b: [128, 2*2*WPAD]
            xdr_b, xe3_b = x_blocks[ib][e]  # xdr_b: [128, 2*2*R]
            for kd in range(2):
                o = OFF_KD[kd]
                ixd[e, :, o : o + WKD] = wdr_b[:, kd * WKD : (kd + 1) * WKD]
                ixd[e, :, o + WKD : o + WKD + XKD] = xdr_b[
                    :, kd * XKD : (kd + 1) * XKD
                ]
            ixd[e, :, OFF_WE3:OFF_XE3] = we3_b
            ixd[e, :, OFF_XE3:] = xe3_b
        blg = np.zeros((2, E * (WPAD + R)), np.float16)
        blg[0, : E * WPAD] = b_rows[ip]
        blg[1, : E * WPAD].reshape(E, WPAD)[:, :PP] = 1.0
        blg[0, E * WPAD :] = 1.0
        blg[1, E * WPAD :] = g_rows[ib]
        in_maps.append({"ixd": ixd, "blg": blg})
    return in_maps


def _install_trace_support():
    """Dev-only plumbing for NTFF profiling under axon: provides the
    antenv.axon_hooks shim this image lacks and disables the S3 artifact
    upload. Returns True if tracing is usable."""
    try:
        import types

        import antenv

        if "antenv.axon_hooks" not in sys.modules:
            mod = types.ModuleType("antenv.axon_hooks")
            mod._hook = None

            def set_axon_ntff_profile_hook(h, _m=mod):
                _m._hook = h

            def get_axon_ntff_profile_hook(_m=mod):
                return _m._hook

            mod.set_axon_ntff_profile_hook = set_axon_ntff_profile_hook
            mod.get_axon_ntff_profile_hook = get_axon_ntff_profile_hook
            sys.modules["antenv.axon_hooks"] = mod
            antenv.axon_hooks = mod

        import antenv.axon_hooks as ah

        if ah.get_axon_ntff_profile_hook() is None:
            from trn_agent_boot.trn_boot import _ntff_profile_via_ctypes

            hook = _ntff_profile_via_ctypes("/opt/axon/libaxon_pjrt.so")
            if hook is None:
                return False
            ah.set_axon_ntff_profile_hook(hook)

        import concourse.bass_utils as bu

        bu.upload_artifacts = lambda tmpdir: f"local:{tmpdir}"
        return True
    except Exception as e:  # pragma: no cover - tracing is best-effort
        print(f"trace support unavailable: {type(e).__name__}: {e}")
        return False


def kernel(**inputs):
    global LAST_RESULT
    from concourse.bass_utils import run_bass_kernel_spmd

    if "nc" not in _CACHE:
        _CACHE["nc"] = _build_nc()
    nc = _CACHE["nc"]

    in_maps = _prep_inputs(inputs)
    trace = os.environ.get("BASS_KERNEL_TRACE", "0") == "1"
    if trace:
        trace = _install_trace_support()
    res = run_bass_kernel_spmd(
        nc, in_maps, core_ids=list(range(NCORES)), trace=trace
    )
    LAST_RESULT = res

    out = np.empty((B, P, C), np.float32)
    for c in range(NCORES):
        ib, ip = divmod(c, PSPLIT)
        # device output is [3, 128, RB*C] p-major
        blk = np.asarray(res.results[c]["out"], np.float32).reshape(3 * 128, RB, C)
        out[ib * RB : (ib + 1) * RB, ip * PP : (ip + 1) * PP, :] = blk[:PP].transpose(
            1, 0, 2
        )
    return out


# revision 25
# speedup vs baseline: 1.1812x; 1.1812x over previous
"""Trainium2 Bass kernel for nn_LinearPredictionHead (moe_routing).

Reference computation:
    out_e = xs_e[:, :, -1, :] @ W_e + b_e            # [B,C,720] per expert
    combined = sum_e gates[:, e, None] * exp(out_e)  # [B,C,720]
    out = log(max(combined, eps)).transpose(0, 2, 1) # [B,720,C]

Sharding (8 cores, no collectives): 2D data-parallel.
  - B=64 split 4 ways (16 batches -> 512 rows of x per core)
  - P=720 split 2 ways (360 output cols -> W cols per core)
  core c: ib = c // 2 (batch group), ip = c % 2 (p half).

Per-core device kernel (p-major, mixed-precision fp8):
  The rel-err gate is 2e-2; all-e4m3 DoubleRow measures 2.2e-2 and all-e3m4
  measures 1.1e-2 (bit-exact host sim; inputs are deterministic).  The mix
  k[0:512) in e4m3 *DoubleRow* (2 passes of K=256 at 2 fp8/cycle) plus
  k[512:1024) in e3m4 (4-mantissa fp8, bf16-speed) lands at 1.75e-2 with
  72 N=512 matmuls/core instead of 96 bf16 ones, and 1-byte input DMA:
    psum[p, r] = sum_k W32[k, p] * x[k, r]    (W pre-scaled by 32; x as-is)
    psum      += b32[p] x 1[r] + 1[p] x lng32[r]   one K=2 fp16 rank-2 MM
                                              (fold bias AND ln(gate): the
                                               exp then needs no bias AP and
                                               no per-expert DVE multiply)
    te  = exp(psum / 32)                      ACT, one wide [128,1536] call
                                              per expert (3 PSUM banks)
    acc += te                                 DVE wide fp16 add
    out = ln(acc) per p-tile, fp16, DMA'd as each tile finalizes.

  Inputs ship as ONE u8 dram block per expert with 7040B contiguous per
  partition (w-e4m3 | x-e4m3 | w-e3m4 | x-e3m4), one dma_start each
  (~0.88MB at near-peak descriptor efficiency); e0's block is split in two
  so its DoubleRow passes start as early as possible.  Framework trims
  carried over from the previous session: combined exp/ln ACT table, the
  init-time all-engine barrier skip, and the slim TileContext exit.
"""

import os
import sys

import numpy as np

if "/opt/trn_rl_repo" not in sys.path:
    sys.path.insert(0, "/opt/trn_rl_repo")

import ml_dtypes

B, C, E = 64, 32, 4
D, P = 1024, 720
NCORES = 8
BSPLIT, PSPLIT = 4, 2
RB = B // BSPLIT  # 16 batches per core
R = RB * C  # 512 rows per core
PP = P // PSPLIT  # 360 output cols per core
PTS = [(0, 128), (128, 128), (256, 104)]  # p-tiles within PP
SCALE = 32.0  # shared psum scale: W quantized as 32*W, x as-is
WPAD = 368  # W free-dim padded so the DoubleRow pair-step is %16
KDR = 512  # k[0:512) via e4m3 DoubleRow, k[512:1024) via e3m4
# per-partition byte offsets inside one expert's combined input block:
# [wdr-kd0 | xdr-kd0 | wdr-kd1 | xdr-kd1 | we3 | xe3] so a kd-granular
# prefix of the block is already usable by the PE (e0 is DMA'd in 3 pieces).
WKD = 2 * WPAD  # 736 one DoubleRow pass of W pairs
XKD = 2 * R  # 1024 one DoubleRow pass of x pairs
OFF_KD = [0, WKD + XKD]  # kd pass bases (w then x inside each)
OFF_WE3 = 2 * (WKD + XKD)  # 3520
OFF_XE3 = OFF_WE3 + 4 * WPAD  # -> 4992
BLK = OFF_XE3 + 4 * R  # [4c,512] e3m4 = 2048 -> 7040
NWARM = 6

_CACHE = {}
LAST_RESULT = None


def _build_nc():
    import concourse.bass as bass_mod
    import concourse.tile as tile
    from concourse import bacc, mybir

    u8 = mybir.dt.uint8
    f16, f32 = mybir.dt.float16, mybir.dt.float32
    f8e4, f8e3 = mybir.dt.float8e4, mybir.dt.float8e3
    DR = mybir.MatmulPerfMode.DoubleRow
    Exp = mybir.ActivationFunctionType.Exp
    Ln = mybir.ActivationFunctionType.Ln

    # Force Exp and Ln onto the combined act-table set so the kernel loads
    # ONE table instead of reloading on every Exp<->Ln switch.
    import concourse.bacc as bacc_mod
    from concourse.hw_specs import get_activation_tables as _orig_gat

    def _patched_gat(arch):
        tables = _orig_gat(arch)
        for name, funcs in tables.items():
            if name != "natural_log_exp_and_others":
                funcs.discard(mybir.ActivationFunctionType.Exp)
                funcs.discard(mybir.ActivationFunctionType.Ln)
        return tables

    bacc_mod.get_activation_tables = _patched_gat

    # Skip the init-time all-engine barrier: it makes every queue wait for
    # the slowest engine preamble before the first user instruction.
    # Nothing emitted before user code (const-AP memsets on gpsimd) is read
    # by this kernel until the Ln (const 0.0 bias) long after; safe here.
    _orig_aeb = bass_mod.Bass.all_engine_barrier
    _state = {"skipped": False}

    def _patched_aeb(self, *a, **k):
        if not _state["skipped"]:
            _state["skipped"] = True
            return
        return _orig_aeb(self, *a, **k)

    bass_mod.Bass.all_engine_barrier = _patched_aeb
    try:
        nc = bacc.Bacc(
            "TRN2", target_bir_lowering=False, debug=False, num_devices=NCORES
        )
    finally:
        bass_mod.Bass.all_engine_barrier = _orig_aeb

    # TileContext exit: drop the exit barrier AND the framework sem clears.
    # The NEFF-load postamble (runtime-injected) starts with its own entry
    # barrier and then resets the whole sem file, so our exit barrier and
    # clears are pure duplication.
    _orig_dab = tile.TileContext._drain_and_barrier

    def _noexit_dab(self, tick_clock, wait_clock):
        # No completion waits either: the output DMAs land during the
        # ~7us runtime postamble (barrier + 250 sem clears), long before
        # the NEFF's done-notify; the postamble entry barrier then fires
        # as soon as each engine's queue drains.
        popped = self.nc._tile_sem_poison_stack.pop()
        assert popped is self._sem_poison

    tile.TileContext._drain_and_barrier = _noexit_dab
    # (Measured: the postamble begins with its own entry barrier, so the
    # clears cannot overlap the kernel; dropping our exit barrier still
    # saves its sem round-trips.)

    # Host-pretiled inputs: one combined block per expert, 7040B/partition
    # contiguous runs; fp16 rank-2 operands (32*b | ones || ones | 32*lng).
    ixd = nc.dram_tensor("ixd", [E, 128, BLK], u8, kind="ExternalInput").ap()
    blg = nc.dram_tensor("blg", [2, E * (WPAD + R)], f16, kind="ExternalInput").ap()
    out = nc.dram_tensor("out", [3, 128, R], f16, kind="ExternalOutput").ap()
    GL0 = E * WPAD  # column where the gl (rhs) rows start inside blg

    with tile.TileContext(nc) as tc:
        with (
            tc.tile_pool(name="const", bufs=1) as cpool,
            tc.tile_pool(name="psum", bufs=7, space="PSUM") as pspool,
            tc.tile_pool(name="warmps", bufs=1, space="PSUM") as wpool,
            tc.tile_pool(name="texp", bufs=3) as tpool,
            tc.tile_pool(name="lnp", bufs=3) as lnpool,
        ):
            warm_t = cpool.tile([128, 512], f16, tag="warm_t")
            nc.vector.memset(warm_t[:], 0.125)

            inb = [
                cpool.tile([128, BLK], u8, tag=f"in{e}", name=f"in{e}")
                for e in range(E)
            ]
            blg_t = cpool.tile([2, E * (WPAD + R)], f16, tag="blg")
            acc = cpool.tile([128, 3 * 512], f16, tag="acc", name="acc")

            # e0's first DoubleRow piece rides the scalar ring ALONE — the
            # two rings' packets interleave round-robin, so anything queued
            # behind the sync stream crawls, but a lone early transfer
            # lands fast.  Everything else rides sync in need-order, led by
            # the tiny blg row (rank-2 operands, needed in the warm window).
            nc.scalar.dma_start(inb[0][:, : OFF_KD[1]], ixd[0, :, : OFF_KD[1]])
            nc.sync.dma_start(blg_t[:], blg[:, :])
            nc.sync.dma_start(
                inb[0][:, OFF_KD[1] : OFF_WE3], ixd[0, :, OFF_KD[1] : OFF_WE3]
            )
            nc.sync.dma_start(inb[0][:, OFF_WE3:], ixd[0, :, OFF_WE3:])
            for e in range(1, E):
                nc.sync.dma_start(inb[e][:], ixd[e])

            # fp8 views into the combined blocks
            wdr, xdr, we3, xe3 = [], [], [], []
            for e in range(E):
                wdr.append(
                    [
                        inb[e][:, OFF_KD[kd] : OFF_KD[kd] + WKD]
                        .bitcast(f8e4)
                        .rearrange("p (i w) -> p i w", i=2)
                        for kd in range(2)
                    ]
                )
                xdr.append(
                    [
                        inb[e][:, OFF_KD[kd] + WKD : OFF_KD[kd] + WKD + XKD]
                        .bitcast(f8e4)
                        .rearrange("p (i r) -> p i r", i=2)
                        for kd in range(2)
                    ]
                )
                we3.append(
                    inb[e][:, OFF_WE3:OFF_XE3]
                    .bitcast(f8e3)
                    .rearrange("p (c w) -> p c w", c=4)
                )
                xe3.append(
                    inb[e][:, OFF_XE3:]
                    .bitcast(f8e3)
                    .rearrange("p (c r) -> p c r", c=4)
                )

            # PE warm-up: dep-free matmuls bridge dispatch->first-data so the
            # HAM clock gate opens before real work lands.
            warm_ps = wpool.tile([128, 512], f32, tag="warm")
            for _ in range(NWARM):
                nc.tensor.matmul(
                    warm_ps[:, :], warm_t[:, :128], warm_t[:, :], start=True, stop=True
                )

            def dr_mm(e, ps, kd, p0, plen, start):
                nc.tensor.matmul(
                    ps[:plen, :],
                    wdr[e][kd][:, :, p0 : p0 + plen],
                    xdr[e][kd][:, :, :],
                    start=start,
                    stop=False,
                    perf_mode=DR,
                )

            def e3_mms(e, ps, p0, plen):
                for c in range(4):
                    nc.tensor.matmul(
                        ps[:plen, :],
                        we3[e][:, c, p0 : p0 + plen],
                        xe3[e][:, c, :],
                        start=False,
                        stop=(c == 3),
                    )

            def rank2(e, ps, plen, p0, start):
                # psum (+)= b32[p] (x) 1[r] + 1[p] (x) lng32[r]; with
                # start=True this OPENS the accumulation group (only the
                # tiny blg row gates it, so it runs pre-data).
                nc.tensor.matmul(
                    ps[:plen, :],
                    blg_t[:, e * WPAD + p0 : e * WPAD + p0 + plen],
                    blg_t[:, GL0 + e * R : GL0 + (e + 1) * R],
                    start=start,
                    stop=False,
                )

            inv = 1.0 / SCALE

            def chain(e, ps, pt):
                # per-p-tile epilogue; single-bank psum tiles keep the next
                # group's matmuls independent of this read.
                sl = slice(512 * pt, 512 * pt + 512)
                if e == 0:
                    nc.scalar.activation(acc[:, sl], ps[:, :], Exp, scale=inv)
                    return
                te = tpool.tile([128, 512], f16, tag="te", name="te")
                nc.scalar.activation(te[:, :], ps[:, :], Exp, scale=inv)
                nc.vector.tensor_add(acc[:, sl], acc[:, sl], te[:, :])
                if e == E - 1:
                    ln_t = lnpool.tile([128, 512], f16, tag="ln")
                    nc.scalar.activation(ln_t[:, :], acc[:, sl], Ln)
                    # stores on the sync ring: ACT's queue then ends at the
                    # last Ln, entering its postamble sooner.
                    nc.sync.dma_start(out[pt], ln_t[:, :])

            # e0's three groups open with their kd0 DoubleRow passes (gated
            # only on the lone scalar-ring piece, in early); then the warm
            # window absorbs 7 rank-2s: e0's own (mid-group) plus openers
            # for e1's groups and e2-pt0.  That's exactly the 7-buffer psum
            # pool — opening more would deadlock the PE FIFO on a buffer
            # freed only by matmuls queued behind the opener.
            ps_t = {}
            for pt, (p0, plen) in enumerate(PTS):
                ps_t[(0, pt)] = pspool.tile([128, 512], f32, tag="ps", name="ps")
                dr_mm(0, ps_t[(0, pt)], 0, p0, plen, start=True)
            for pt, (p0, plen) in enumerate(PTS):
                rank2(0, ps_t[(0, pt)], plen, p0, start=False)
            for e, pt in [(1, 0), (1, 1), (1, 2), (2, 0)]:
                p0, plen = PTS[pt]
                ps_t[(e, pt)] = pspool.tile([128, 512], f32, tag="ps", name="ps")
                rank2(e, ps_t[(e, pt)], plen, p0, start=True)
            # e0 remainder: kd1 passes, then e3m4 + chain per p-tile.
            for pt, (p0, plen) in enumerate(PTS):
                dr_mm(0, ps_t[(0, pt)], 1, p0, plen, start=False)
            for pt, (p0, plen) in enumerate(PTS):
                e3_mms(0, ps_t[(0, pt)], p0, plen)
                chain(0, ps_t[(0, pt)], pt)
            # e1..e3: p-tile-sequential groups, each chained immediately.
            for e in range(1, E):
                for pt, (p0, plen) in enumerate(PTS):
                    ps = ps_t.get((e, pt))
                    if ps is None:
                        ps = pspool.tile([128, 512], f32, tag="ps", name="ps")
                        rank2(e, ps, PTS[pt][1], PTS[pt][0], start=True)
                    dr_mm(e, ps, 0, p0, plen, start=False)
                    dr_mm(e, ps, 1, p0, plen, start=False)
                    e3_mms(e, ps, p0, plen)
                    chain(e, ps, pt)

    tile.TileContext._drain_and_barrier = _orig_dab
    nc.compile()
    return nc


def _q4(v):
    return np.clip(v, -240.0, 240.0).astype(ml_dtypes.float8_e4m3)


def _q3(v):
    return np.clip(v, -15.5, 15.5).astype(ml_dtypes.float8_e3m4)


def _prep_inputs(inputs):
    gates = np.asarray(inputs["gates"], dtype=np.float64)

    # Per p-half, per expert: W byte blocks [128, 1472] (e4m3 DR) and
    # [128, 1472] (e3m4), plus the fp16 rank-2 lhsT rows.
    w_blocks = []  # [ip][e] -> (wdr_bytes, we3_bytes)
    b_rows = []  # [ip] -> [E*WPAD] fp16 row of 32*b
    for ip in range(PSPLIT):
        per_e = []
        brow = np.zeros(E * WPAD, np.float16)
        for e in range(E):
            W32 = (
                np.asarray(inputs[f"W{e}"][:, ip * PP : (ip + 1) * PP], np.float32)
                * SCALE
            )
            wdr = np.zeros((128, 2, 2, WPAD), ml_dtypes.float8_e4m3)
            wdr[:, :, :, :PP] = _q4(
                W32[:KDR].reshape(2, 2, 128, PP).transpose(2, 0, 1, 3)
            )
            we3 = np.zeros((128, 4, WPAD), ml_dtypes.float8_e3m4)
            we3[:, :, :PP] = _q3(W32[KDR:].reshape(4, 128, PP).transpose(1, 0, 2))
            per_e.append(
                (
                    wdr.view(np.uint8).reshape(128, -1),
                    we3.view(np.uint8).reshape(128, -1),
                )
            )
            brow[e * WPAD : e * WPAD + PP] = (
                SCALE * np.asarray(inputs[f"b{e}"][ip * PP : (ip + 1) * PP])
            ).astype(np.float16)
        w_blocks.append(per_e)
        b_rows.append(brow)

    # Per b-group, per expert: x byte blocks and the lng rank-2 rhs rows.
    x_blocks = []  # [ib][e] -> (xdr_bytes, xe3_bytes)
    g_rows = []  # [ib] -> [E*R] fp16 row of 32*ln(g)
    for ib in range(BSPLIT):
        per_e = []
        grow = np.empty(E * R, np.float16)
        for e in range(E):
            xl = np.asarray(
                inputs[f"xs{e}"][ib * RB : (ib + 1) * RB, :, -1, :], np.float32
            ).reshape(R, D)
            xdr = _q4(xl[:, :KDR].reshape(R, 2, 2, 128).transpose(3, 1, 2, 0))
            xe3 = _q3(xl[:, KDR:].reshape(R, 4, 128).transpose(2, 1, 0))
            per_e.append(
                (
                    np.ascontiguousarray(xdr).view(np.uint8).reshape(128, -1),
                    np.ascontiguousarray(xe3).view(np.uint8).reshape(128, -1),
                )
            )
            lng = SCALE * np.log(np.maximum(gates[ib * RB : (ib + 1) * RB, e], 1e-6))
            grow[e * R : (e + 1) * R] = np.repeat(lng, C).astype(np.float16)
        x_blocks.append(per_e)
        g_rows.append(grow)

    in_maps = []
    for c in range(NCORES):
        ib, ip = divmod(c, PSPLIT)
        ixd = np.empty((E, 128, BLK), np.uint8)
        for e in range(E):
            wdr_b, we3_b = w_blocks[ip][e]  # wdr_b: [128, 2*2*WPAD]
            xdr_b, xe3_b = x_blocks[ib][e]  # xdr_b: [128, 2*2*R]
            for kd in range(2):
                o = OFF_KD[kd]
                ixd[e, :, o : o + WKD] = wdr_b[:, kd * WKD : (kd + 1) * WKD]
                ixd[e, :, o + WKD : o + WKD + XKD] = xdr_b[
                    :, kd * XKD : (kd + 1) * XKD
                ]
            ixd[e, :, OFF_WE3:OFF_XE3] = we3_b
            ixd[e, :, OFF_XE3:] = xe3_b
        blg = np.zeros((2, E * (WPAD + R)), np.float16)
        blg[0, : E * WPAD] = b_rows[ip]
        blg[1, : E * WPAD].reshape(E, WPAD)[:, :PP] = 1.0
        blg[0, E * WPAD :] = 1.0
        blg[1, E * WPAD :] = g_rows[ib]
        in_maps.append({"ixd": ixd, "blg": blg})
    return in_maps


def _install_trace_support():
    """Dev-only plumbing for NTFF profiling under axon: provides the
    antenv.axon_hooks shim this image lacks and disables the S3 artifact
    upload. Returns True if tracing is usable."""
    try:
        import types

        import antenv

        if "antenv.axon_hooks" not in sys.modules:
            mod = types.ModuleType("antenv.axon_hooks")
            mod._hook = None

            def set_axon_ntff_profile_hook(h, _m=mod):
                _m._hook = h

            def get_axon_ntff_profile_hook(_m=mod):
                return _m._hook

            mod.set_axon_ntff_profile_hook = set_axon_ntff_profile_hook
            mod.get_axon_ntff_profile_hook = get_axon_ntff_profile_hook
            sys.modules["antenv.axon_hooks"] = mod
            antenv.axon_hooks = mod

        import antenv.axon_hooks as ah

        if ah.get_axon_ntff_profile_hook() is None:
            from trn_agent_boot.trn_boot import _ntff_profile_via_ctypes

            hook = _ntff_profile_via_ctypes("/opt/axon/libaxon_pjrt.so")
            if hook is None:
                return False
            ah.set_axon_ntff_profile_hook(hook)

        import concourse.bass_utils as bu

        bu.upload_artifacts = lambda tmpdir: f"local:{tmpdir}"
        return True
    except Exception as e:  # pragma: no cover - tracing is best-effort
        print(f"trace support unavailable: {type(e).__name__}: {e}")
        return False


def kernel(**inputs):
    global LAST_RESULT
    from concourse.bass_utils import run_bass_kernel_spmd

    if "nc" not in _CACHE:
        _CACHE["nc"] = _build_nc()
    nc = _CACHE["nc"]

    in_maps = _prep_inputs(inputs)
    trace = os.environ.get("BASS_KERNEL_TRACE", "0") == "1"
    if trace:
        trace = _install_trace_support()
    res = run_bass_kernel_spmd(
        nc, in_maps, core_ids=list(range(NCORES)), trace=trace
    )
    LAST_RESULT = res

    out = np.empty((B, P, C), np.float32)
    for c in range(NCORES):
        ib, ip = divmod(c, PSPLIT)
        # device output is [3, 128, RB*C] p-major
        blk = np.asarray(res.results[c]["out"], np.float32).reshape(3 * 128, RB, C)
        out[ib * RB : (ib + 1) * RB, ip * PP : (ip + 1) * PP, :] = blk[:PP].transpose(
            1, 0, 2
        )
    return out
